# revision 78
# baseline (speedup 1.0000x reference)
"""Multi-head attention (dense transformer block) on 8 Trainium2 NeuronCores.

Sharding: (batch=4) x (head-half=2) -> 8 shards. Each core computes, for its
batch element b and its 8 heads (heads 8*hh .. 8*hh+7), the FULL 2048-query
attention plus a PARTIAL output projection (contraction over its 512 head
dims); the host-side gather sums the two partials per batch element (the
output projection is linear; the bias is folded into the hh=0 core only).
This removes the redundant K/V projection compute the pure data-parallel
split required, with zero cross-core communication.

Per core:
  Qt_p/Kt_p = (W @ x.T)[pair rows]  (transposed, 128 x 2048) for 4 head-pairs
  V         = x @ Wv.T (natural layout, per-head 65-col interleave with a
              trailing ones column for the softmax denominator)
  attention per pair p, query-block qb (512 q), key-chunk k (128 keys):
    St for both heads in one 2-bank PSUM tile via a row-paired matmul pair,
    one wide exp(St/8) on ACT, and one M=65 matmul per head accumulates
    [V.T @ Pt ; ones.T @ Pt] so numerator and denominator come from the
    same instruction.
    normalize: 1/Z via a fast approximate DVE reciprocal; the 64-partition
    broadcast (tiny ones outer-product matmuls) for block qb is DEFERRED
    into the next block's k-loop, so the tensor queue never waits on the
    reciprocal (the baseline lost ~70us to that head-of-line stall).
  out_partial = Ot.T @ Wo[pair rows] + (bo if hh==0)   [2048 x 1024 f32]

Emission order pipelines the engines: V-projection PSUM groups interleave
into the first attention query block, Q/K projections for pair p+1
interleave per query block of pair p, and output-projection chunks
interleave into the last pair's attention, so the tensor engine never
idles waiting on ACT and vice versa.

Matmul operands are bf16 (fp32 PSUM accumulation).
"""

import numpy as np
import ml_dtypes

import concourse.bass as bass
import concourse.tile as tile
import concourse.mybir as mybir
from concourse.bass_utils import run_bass_kernel_spmd

F32 = mybir.dt.float32
F32R = mybir.dt.float32r
BF16 = mybir.dt.bfloat16
EXP = mybir.ActivationFunctionType.Exp
IDENT = mybir.ActivationFunctionType.Identity

D = 1024          # d_model
S = 2048          # sequence length
NH = 16           # heads total
DH = 64           # head dim
HD2 = 512         # head dims per core (8 heads)
NP = 4            # head-pairs per core
NCORES = 8
VERSION = 22       # bump on every program change: busts stale NEFF caches


def split_multi_waits(nc):
    """The walrus build in this container accepts at most one sync-wait per
    instruction; move extra waits onto same-engine nops inserted before the
    offending instruction."""
    k = 0
    for f in nc.m.functions:
        for bb in f.blocks:
            out, changed = [], False
            for inst in bb.instructions:
                si = inst.sync_info
                waits = list(si.on_wait) if si and si.on_wait else []
                if len(waits) > 1:
                    changed = True
                    for w in waits[:-1]:
                        nop = mybir.InstNoOp(name=f"wsplit-{k}", ins=[], outs=[])
                        k += 1
                        nop.engine = inst.engine
                        nop.sync_info = mybir.SyncInfo(on_wait=[w], on_update=[])
                        nc.register_instruction(nop, overwrite=True)
                        out.append(nop)
                    si.on_wait = waits[-1:]
                out.append(inst)
            if changed:
                bb.instructions = out


def build_program():
    nc = bass.Bass()
    xT = nc.declare_dram_parameter("xT", [D, S], BF16, isOutput=False)
    wqh = nc.declare_dram_parameter("wqh", [D, HD2], BF16, isOutput=False)
    wkh = nc.declare_dram_parameter("wkh", [D, HD2], BF16, isOutput=False)
    wvh = nc.declare_dram_parameter("wvh", [D, HD2], BF16, isOutput=False)
    woh = nc.declare_dram_parameter("woh", [HD2, D], BF16, isOutput=False)
    bq2 = nc.declare_dram_parameter("bq2", [128, NP], F32, isOutput=False)
    bk2 = nc.declare_dram_parameter("bk2", [128, NP], F32, isOutput=False)
    bvb = nc.declare_dram_parameter("bvb", [128, HD2], F32, isOutput=False)
    bob = nc.declare_dram_parameter("bob", [128, D], F32, isOutput=False)
    ones2 = nc.declare_dram_parameter("ones2", [2, 64], BF16, isOutput=False)
    tag = nc.declare_dram_parameter("tag", [1, VERSION], F32, isOutput=False)
    out = nc.declare_dram_parameter("out", [S, D], BF16, isOutput=True)

    with tile.TileContext(nc) as tc:
        with tc.tile_pool(name="persist", bufs=1) as pp, \
             tc.tile_pool(name="qk", bufs=2) as qkp, \
             tc.tile_pool(name="pt", bufs=4) as ptp, \
             tc.tile_pool(name="rz", bufs=2) as rzp, \
             tc.tile_pool(name="osb", bufs=2) as op_, \
             tc.tile_pool(name="psS", bufs=2, space="PSUM") as stp, \
             tc.tile_pool(name="psO", bufs=2, space="PSUM") as pop:
            tag_sb = pp.tile([1, VERSION], F32, name="tag_sb", tag="tag_sb")
            nc.sync.dma_start(tag_sb[:], tag[:])
            # ones rows at partitions 0 and 32 (matmul lhsT base must match
            # its rhs base; the 1/Z rows live at partitions 0 and 32)
            ones_sb = pp.tile([33, 64], BF16, name="ones_sb", tag="ones_sb")
            nc.sync.dma_start(ones_sb[0:1, :], ones2[0:1, :])
            nc.sync.dma_start(ones_sb[32:33, :], ones2[1:2, :])

            # resident tiles
            xt_sb = [pp.tile([128, S], BF16, name=f"xt{d}", tag=f"xt{d}")
                     for d in range(8)]
            wq_sb = [pp.tile([128, HD2], BF16, name=f"wq{d}", tag=f"wq{d}")
                     for d in range(8)]
            wk_sb = [pp.tile([128, HD2], BF16, name=f"wk{d}", tag=f"wk{d}")
                     for d in range(8)]
            wv_sb = [pp.tile([128, HD2], BF16, name=f"wv{d}", tag=f"wv{d}")
                     for d in range(8)]
            wo_sb = [pp.tile([128, D], BF16, name=f"wo{d}", tag=f"wo{d}")
                     for d in range(4)]
            vg = [pp.tile([128, 8 * (DH + 1)], BF16, name=f"vg{t}", tag=f"vg{t}")
                  for t in range(16)]
            ot = [pp.tile([128, S], BF16, name=f"ot{p}", tag=f"ot{p}")
                  for p in range(NP)]
            bq_sb = pp.tile([128, NP], F32, name="bq_sb", tag="bq_sb")
            bk_sb = pp.tile([128, NP], F32, name="bk_sb", tag="bk_sb")
            bvb_sb = pp.tile([128, HD2], F32, name="bvb_sb", tag="bvb_sb")
            bob_sb = pp.tile([128, D], F32, name="bob_sb", tag="bob_sb")

            # DMA issue order = priority order. HWDGE setup is ~0.6us per
            # dma_start on the issuing sequencer, so balance the 56 input
            # DMAs across BOTH capable engines (sync + scalar) or the issue
            # serialization itself (~24us) becomes the prologue bottleneck.
            for d in range(8):
                nc.sync.dma_start(wq_sb[d][:], wqh[128 * d:128 * (d + 1), :])
                nc.scalar.dma_start(wk_sb[d][:], wkh[128 * d:128 * (d + 1), :])
            for d in range(8):
                nc.sync.dma_start(xt_sb[d][:, 0:512], xT[128 * d:128 * (d + 1), 0:512])
                nc.scalar.dma_start(xt_sb[d][:, 1024:1536],
                                    xT[128 * d:128 * (d + 1), 1024:1536])
            for d in range(8):
                nc.sync.dma_start(xt_sb[d][:, 512:1024],
                                  xT[128 * d:128 * (d + 1), 512:1024])
                nc.scalar.dma_start(xt_sb[d][:, 1536:2048],
                                    xT[128 * d:128 * (d + 1), 1536:2048])
            for d in range(8):
                nc.scalar.dma_start(wv_sb[d][:], wvh[128 * d:128 * (d + 1), :])
            # tiny biases early on sync; the big late-needed tensors (bvb,
            # bob, wo) at the END of scalar's queue so they don't steal
            # early DMA bandwidth from the critical wq/wk/xt loads (gpsimd
            # issues start first and would front-run them).
            nc.sync.dma_start(bq_sb[:], bq2[:])
            nc.sync.dma_start(bk_sb[:], bk2[:])
            nc.scalar.dma_start(bvb_sb[:], bvb[:])
            nc.scalar.dma_start(bob_sb[:], bob[:])
            for d in range(4):
                nc.scalar.dma_start(wo_sb[d][:], woh[128 * d:128 * (d + 1), :])

            def proj_group(w_sb, p, qh, dst, b_sb):
                """One [128,1024] PSUM group: token cols 1024*qh..+1024 of the
                pair-p projection, written transposed into dst (qt/kt).
                Returns the written AP (used as a scheduling guard)."""
                ps = stp.tile([128, 1024], F32, name="psp", tag="st", bufs=2)
                for half in range(2):
                    qs = slice(1024 * qh + 512 * half, 1024 * qh + 512 * (half + 1))
                    for d in range(8):
                        nc.tensor.matmul(
                            ps[:, 512 * half:512 * (half + 1)],
                            w_sb[d][:, 128 * p:128 * (p + 1)],
                            xt_sb[d][:, qs],
                            start=(d == 0), stop=(d == 7))
                out_ap = dst[:, 1024 * qh:1024 * (qh + 1)]
                nc.vector.tensor_scalar_add(out_ap, ps[:], b_sb[:, p:p + 1])
                return out_ap

            def v_group(g):
                """V projection for token chunks ti=2g, 2g+1 in one PSUM tile."""
                ps = stp.tile([128, 1024], F32, name="psv", tag="st", bufs=2)
                for half in range(2):
                    ti = 2 * g + half
                    for d in range(8):
                        nc.tensor.matmul(
                            ps[:, 512 * half:512 * (half + 1)],
                            xt_sb[d][:, 128 * ti:128 * (ti + 1)],
                            wv_sb[d][:, :],
                            start=(d == 0), stop=(d == 7))
                for half in range(2):
                    ti = 2 * g + half
                    dst = vg[ti][:].rearrange("p (h w) -> p h w", w=65)[:, :, 0:64]
                    nc.vector.tensor_add(
                        dst,
                        ps[:, 512 * half:512 * (half + 1)].rearrange(
                            "p (h w) -> p h w", w=64),
                        bvb_sb[:].rearrange("p (h w) -> p h w", w=64))
                    nc.vector.memset(
                        vg[ti][:].rearrange("p (h w) -> p h w", w=65)[:, :, 64:65],
                        1.0)

            def o_group(t8, split=1):
                """Output projection for token rows 128*t8..+128 (partial over
                this core's 512 head dims), + bias, DMA to DRAM. The tail
                groups pass split=2 so the final transfers drain on two
                queues each."""
                ps = stp.tile([128, 1024], F32, name="pso", tag="st", bufs=2)
                for hf in range(2):
                    for pp_ in range(NP):
                        nc.tensor.matmul(
                            ps[:, 512 * hf:512 * (hf + 1)],
                            ot[pp_][:, 128 * t8:128 * (t8 + 1)],
                            wo_sb[pp_][:, 512 * hf:512 * (hf + 1)],
                            start=(pp_ == 0), stop=(pp_ == NP - 1))
                osb = op_.tile([128, D], BF16, name="osb", tag="osb", bufs=2)
                nc.vector.tensor_add(osb[:], ps[:], bob_sb[:])
                for c in range(split):
                    cs = slice(D // split * c, D // split * (c + 1))
                    nc.sync.dma_start(out[128 * t8:128 * (t8 + 1), cs],
                                      osb[:, cs])

            # Q/K projections for pairs 0 and 1 up front (first tensor work).
            qt = [None] * NP
            kt = [None] * NP

            def qk_proj_groups(p):
                """Return the 4 emission thunks of pair p's Q+K projections."""
                qt[p] = qkp.tile([128, S], BF16, name=f"qt_p{p}", tag="qt", bufs=2)
                kt[p] = qkp.tile([128, S], BF16, name=f"kt_p{p}", tag="kt", bufs=2)
                return [
                    lambda qh=qh: proj_group(wq_sb, p, qh, qt[p], bq_sb)
                    for qh in range(2)
                ] + [
                    lambda qh=qh: proj_group(wk_sb, p, qh, kt[p], bk_sb)
                    for qh in range(2)
                ]

            for g in qk_proj_groups(0):
                g()
            pending1 = qk_proj_groups(1)
            for g in pending1[:2]:
                g()

            def finish_norm(p, qb, po, rzb):
                """Broadcast 1/Z down 64 partitions (tiny bf16 ones
                outer-product matmuls into an st-pool PSUM tile), copy to
                SBUF, multiply. Emitted ~2 k-iterations into the NEXT
                attention block so the tensor queue never waits on the
                reciprocal."""
                qs = slice(512 * qb, 512 * (qb + 1))
                pb = stp.tile([128, 1024], F32, name="pb", tag="st", bufs=2)
                nc.tensor.matmul(pb[0:64, 0:512], ones_sb[0:1, :],
                                 rzb[0:1, 0:512], start=True, stop=True)
                nc.tensor.matmul(pb[0:64, 512:1024], ones_sb[32:33, :],
                                 rzb[32:33, 512:1024], start=True, stop=True)
                rb = rzp.tile([64, 1024], F32, name="rb", tag="rb", bufs=2)
                nc.vector.tensor_copy(rb[:], pb[0:64, :])
                nc.vector.tensor_mul(ot[p][0:64, qs], po[0:64, 0:512],
                                     rb[0:64, 0:512])
                nc.vector.tensor_mul(ot[p][64:128, qs], po[0:64, 512:1024],
                                     rb[0:64, 512:1024])

            def attention(p, qb, filler, norm_prev):
                """Attention for pair p, query block qb (512 queries).
                `filler`: list of (k_slot, thunk) emission thunks
                (projection / output groups) interleaved into the k-loop to
                keep the tensor queue fed.
                `norm_prev`: (recip_thunk, finish_thunk) of the previous
                block. The reciprocal goes at k==3 so it sits BEHIND this
                block's projection bias-add on the in-order DVE queue (a
                recip emitted earlier delays that add and stalls the tensor
                engine on the st-pool slot); the broadcast+multiply go at
                k==12, by which time the reciprocal is long done.
                Returns this block's (recip_thunk, finish_thunk)."""
                c0 = 130 * p
                c1 = 130 * p + 65
                qs = slice(512 * qb, 512 * (qb + 1))
                po = pop.tile([128, 1024], F32, name="po", tag="po", bufs=2)
                filler = sorted(filler, key=lambda x: x[0])
                fi = 0
                guard = None
                for k in range(16):
                    # finish_norm MUST precede same-slot fillers (the p==3
                    # output-projection groups read ot written by its muls)
                    if norm_prev is not None and k == 12:
                        norm_prev[1]()
                    while fi < len(filler) and filler[fi][0] == k:
                        r = filler[fi][1]()
                        if r is not None:
                            guard = r
                        fi += 1
                    # ... while the reciprocal guard needs the k<=3 filler
                    # (projection group) emitted first
                    if norm_prev is not None and k == 3:
                        norm_prev[0](guard)
                    ks = slice(128 * k, 128 * (k + 1))
                    st = stp.tile([128, 1024], F32, name="st", tag="st", bufs=2)
                    nc.tensor.matmul(st[:, 0:512], kt[p][0:64, ks], qt[p][0:64, qs],
                                     start=True, stop=True)
                    nc.tensor.matmul(st[:, 512:1024], kt[p][64:128, ks],
                                     qt[p][64:128, qs], start=True, stop=True)
                    pt = ptp.tile([128, 1024], BF16, name="pt", tag="pt", bufs=4)
                    nc.scalar.activation(pt[:], st[:], EXP, scale=0.125)
                    first, last = (k == 0), (k == 15)
                    nc.tensor.matmul(po[0:65, 0:512], vg[k][:, c0:c0 + 65],
                                     pt[:, 0:512], start=first, stop=last)
                    nc.tensor.matmul(po[0:65, 512:1024], vg[k][:, c1:c1 + 65],
                                     pt[:, 512:1024], start=first, stop=last)
                while fi < len(filler):
                    filler[fi][1]()
                    fi += 1
                rz = rzp.tile([33, 1024], F32, name="rz", tag="rz", bufs=2)
                rzb = rzp.tile([33, 1024], BF16, name="rzb", tag="rzb", bufs=2)

                def recip_thunk(guard_ap=None):
                    # The per-engine schedule is STATIC, ordered by
                    # dependency readiness -- the ~7us reciprocal chain
                    # would get placed ahead of the co-scheduled projection
                    # group's PSUM drain on DVE, and the held st-pool slot
                    # then stalls the tensor engine ~6us. Guard: copy the
                    # drained projection row into po's unused PSUM row 96
                    # (partition bases must be 32-aligned); each recip chunk
                    # reads rows 64:97, giving every chunk a clean RAW
                    # dependency on the drain. (rz rows 1..32 are
                    # recip-of-garbage and never read.)
                    with nc.allow_low_precision(reason="softmax 1/Z"):
                        if guard_ap is not None:
                            nc.vector.tensor_copy(po[96:97, :],
                                                  guard_ap[0:1, :])
                        for i in range(8):
                            cs = slice(128 * i, 128 * (i + 1))
                            if guard_ap is not None:
                                nc.vector.reciprocal(rz[0:33, cs],
                                                     po[64:97, cs])
                            else:
                                nc.vector.reciprocal(rz[0:1, cs],
                                                     po[64:65, cs])
                    nc.vector.tensor_copy(rzb[0:1, 0:512], rz[0:1, 0:512])
                    nc.vector.tensor_copy(rzb[32:33, 512:1024],
                                          rz[0:1, 512:1024])

                return (recip_thunk, lambda: finish_norm(p, qb, po, rzb))

            # pair 0, query block 0: interleave the 8 V-projection groups
            # (V group g covers key chunks 2g,2g+1, always ahead of PV k=g).
            norm = attention(0, 0,
                             [(2 * g, lambda g=g: v_group(g)) for g in range(8)],
                             None)
            # pair 0, blocks 1-3: finish pair 1's projections early
            # proj bursts at slot 2, not 0: St(k0)/St(k1) queue ~2 EXPs of ACT
            # backlog first, covering most of the 3.5us tensor burst
            norm = attention(0, 1, [(2, pending1[2]), (8, pending1[3])], norm)
            norm = attention(0, 2, [], norm)
            norm = attention(0, 3, [], norm)
            # pairs 1..3: interleave next pair's projections per query block
            for p in range(1, NP):
                nxt = qk_proj_groups(p + 1) if p + 1 < NP else []
                for qb in range(4):
                    filler = [(2, nxt[qb])] if qb < len(nxt) else []
                    # last pair: interleave output projection for finished
                    # query blocks (needs all pairs => only valid on p==3;
                    # slots 12+ put them after the deferred normalize)
                    if p == NP - 1 and qb > 0:
                        filler = [(12 + i, lambda t8=t8: o_group(t8))
                                  for i, t8 in enumerate(range(4 * (qb - 1), 4 * qb))]
                    norm = attention(p, qb, filler, norm)
            # tail: final normalize + output projection for the last block
            norm[0]()
            norm[1]()
            for t8 in range(12, 16):
                o_group(t8, split=2)

    split_multi_waits(nc)
    return nc


_CACHED_NC = None


def get_program():
    global _CACHED_NC
    if _CACHED_NC is None:
        _CACHED_NC = build_program()
    return _CACHED_NC


def make_in_maps(x, Wq, bq, Wk, bk, Wv, bv, Wo, bo):
    x = np.asarray(x, np.float32)
    bf = ml_dtypes.bfloat16
    wqT = np.ascontiguousarray(np.asarray(Wq, np.float32).T)
    wkT = np.ascontiguousarray(np.asarray(Wk, np.float32).T)
    wvT = np.ascontiguousarray(np.asarray(Wv, np.float32).T)
    woT = np.ascontiguousarray(np.asarray(Wo, np.float32).T)
    bq = np.asarray(bq, np.float32)
    bk = np.asarray(bk, np.float32)
    bv = np.asarray(bv, np.float32)
    bo = np.asarray(bo, np.float32)
    in_maps = []
    for c in range(NCORES):
        b, hh = c // 2, c % 2
        hs = slice(HD2 * hh, HD2 * (hh + 1))
        m = {
            "xT": np.ascontiguousarray(x[b].T).astype(bf),
            "wqh": np.ascontiguousarray(wqT[:, hs]).astype(bf),
            "wkh": np.ascontiguousarray(wkT[:, hs]).astype(bf),
            "wvh": np.ascontiguousarray(wvT[:, hs]).astype(bf),
            "woh": np.ascontiguousarray(woT[hs, :]).astype(bf),
            "bq2": np.ascontiguousarray(bq[hs].reshape(NP, 128).T),
            "bk2": np.ascontiguousarray(bk[hs].reshape(NP, 128).T),
            "bvb": np.ascontiguousarray(np.tile(bv[hs], (128, 1))),
            "bob": np.ascontiguousarray(
                np.tile(bo if hh == 0 else np.zeros_like(bo), (128, 1))),
            "ones2": np.ones((2, 64), ml_dtypes.bfloat16),
            "tag": np.zeros((1, VERSION), np.float32),
        }
        in_maps.append(m)
    return in_maps


def kernel(x, Wq, bq, Wk, bk, Wv, bv, Wo, bo):
    nc = get_program()
    in_maps = make_in_maps(x, Wq, bq, Wk, bk, Wv, bv, Wo, bo)
    res = run_bass_kernel_spmd(nc, in_maps, list(range(NCORES)))
    out = np.empty((4, S, D), np.float32)
    for b in range(4):
        out[b] = np.asarray(res.results[2 * b]["out"], np.float32)
        out[b] += np.asarray(res.results[2 * b + 1]["out"], np.float32)
    return out


# revision 79
# speedup vs baseline: 1.0067x; 1.0067x over previous
"""Multi-head attention (dense transformer block) on 8 Trainium2 NeuronCores.

Sharding: (batch=4) x (head-half=2) -> 8 shards. Each core computes, for its
batch element b and its 8 heads (heads 8*hh .. 8*hh+7), the FULL 2048-query
attention plus a PARTIAL output projection (contraction over its 512 head
dims); the host-side gather sums the two partials per batch element (the
output projection is linear; the bias is folded into the hh=0 core only).
This removes the redundant K/V projection compute the pure data-parallel
split required, with zero cross-core communication.

Per core:
  Qt_p/Kt_p = (W @ x.T)[pair rows]  (transposed, 128 x 2048) for 4 head-pairs
  V         = x @ Wv.T (natural layout, per-head 65-col interleave with a
              trailing ones column for the softmax denominator)
  attention per pair p, query-block qb (512 q), key-chunk k (128 keys):
    St for both heads in one 2-bank PSUM tile via a row-paired matmul pair,
    one wide exp(St/8) on ACT, and one M=65 matmul per head accumulates
    [V.T @ Pt ; ones.T @ Pt] so numerator and denominator come from the
    same instruction.
    normalize: 1/Z via a fast approximate DVE reciprocal; the 64-partition
    broadcast (tiny ones outer-product matmuls) for block qb is DEFERRED
    into the next block's k-loop, so the tensor queue never waits on the
    reciprocal (the baseline lost ~70us to that head-of-line stall).
  out_partial = Ot.T @ Wo[pair rows] + (bo if hh==0)   [2048 x 1024 f32]

Emission order pipelines the engines: V-projection PSUM groups interleave
into the first attention query block, Q/K projections for pair p+1
interleave per query block of pair p, and output-projection chunks
interleave into the last pair's attention, so the tensor engine never
idles waiting on ACT and vice versa.

Matmul operands are bf16 (fp32 PSUM accumulation).
"""

import numpy as np
import ml_dtypes

import concourse.bass as bass
import concourse.tile as tile
import concourse.mybir as mybir
from concourse.bass_utils import run_bass_kernel_spmd

F32 = mybir.dt.float32
F32R = mybir.dt.float32r
BF16 = mybir.dt.bfloat16
EXP = mybir.ActivationFunctionType.Exp
IDENT = mybir.ActivationFunctionType.Identity

D = 1024          # d_model
S = 2048          # sequence length
NH = 16           # heads total
DH = 64           # head dim
HD2 = 512         # head dims per core (8 heads)
NP = 4            # head-pairs per core
NCORES = 8
VERSION = 23       # bump on every program change: busts stale NEFF caches


def split_multi_waits(nc):
    """The walrus build in this container accepts at most one sync-wait per
    instruction; move extra waits onto same-engine nops inserted before the
    offending instruction."""
    k = 0
    for f in nc.m.functions:
        for bb in f.blocks:
            out, changed = [], False
            for inst in bb.instructions:
                si = inst.sync_info
                waits = list(si.on_wait) if si and si.on_wait else []
                if len(waits) > 1:
                    changed = True
                    for w in waits[:-1]:
                        nop = mybir.InstNoOp(name=f"wsplit-{k}", ins=[], outs=[])
                        k += 1
                        nop.engine = inst.engine
                        nop.sync_info = mybir.SyncInfo(on_wait=[w], on_update=[])
                        nc.register_instruction(nop, overwrite=True)
                        out.append(nop)
                    si.on_wait = waits[-1:]
                out.append(inst)
            if changed:
                bb.instructions = out


def build_program():
    nc = bass.Bass()
    xT = nc.declare_dram_parameter("xT", [D, S], BF16, isOutput=False)
    wqh = nc.declare_dram_parameter("wqh", [D, HD2], BF16, isOutput=False)
    wkh = nc.declare_dram_parameter("wkh", [D, HD2], BF16, isOutput=False)
    wvh = nc.declare_dram_parameter("wvh", [D, HD2], BF16, isOutput=False)
    woh = nc.declare_dram_parameter("woh", [HD2, D], BF16, isOutput=False)
    bq2 = nc.declare_dram_parameter("bq2", [128, NP], F32, isOutput=False)
    bk2 = nc.declare_dram_parameter("bk2", [128, NP], F32, isOutput=False)
    bvb = nc.declare_dram_parameter("bvb", [128, HD2], F32, isOutput=False)
    bob = nc.declare_dram_parameter("bob", [128, D], F32, isOutput=False)
    ones2 = nc.declare_dram_parameter("ones2", [2, 64], BF16, isOutput=False)
    tag = nc.declare_dram_parameter("tag", [1, VERSION], F32, isOutput=False)
    out = nc.declare_dram_parameter("out", [S, D], BF16, isOutput=True)

    with tile.TileContext(nc) as tc:
        with tc.tile_pool(name="persist", bufs=1) as pp, \
             tc.tile_pool(name="qk", bufs=2) as qkp, \
             tc.tile_pool(name="pt", bufs=4) as ptp, \
             tc.tile_pool(name="rz", bufs=2) as rzp, \
             tc.tile_pool(name="osb", bufs=2) as op_, \
             tc.tile_pool(name="psS", bufs=2, space="PSUM") as stp, \
             tc.tile_pool(name="psO", bufs=2, space="PSUM") as pop:
            tag_sb = pp.tile([1, VERSION], F32, name="tag_sb", tag="tag_sb")
            nc.sync.dma_start(tag_sb[:], tag[:])
            # ones rows at partitions 0 and 32 (matmul lhsT base must match
            # its rhs base; the 1/Z rows live at partitions 0 and 32)
            ones_sb = pp.tile([33, 64], BF16, name="ones_sb", tag="ones_sb")
            nc.sync.dma_start(ones_sb[0:1, :], ones2[0:1, :])
            nc.sync.dma_start(ones_sb[32:33, :], ones2[1:2, :])

            # resident tiles
            xt_sb = [pp.tile([128, S], BF16, name=f"xt{d}", tag=f"xt{d}")
                     for d in range(8)]
            wq_sb = [pp.tile([128, HD2], BF16, name=f"wq{d}", tag=f"wq{d}")
                     for d in range(8)]
            wk_sb = [pp.tile([128, HD2], BF16, name=f"wk{d}", tag=f"wk{d}")
                     for d in range(8)]
            wv_sb = [pp.tile([128, HD2], BF16, name=f"wv{d}", tag=f"wv{d}")
                     for d in range(8)]
            wo_sb = [pp.tile([128, D], BF16, name=f"wo{d}", tag=f"wo{d}")
                     for d in range(4)]
            vg = [pp.tile([128, 8 * (DH + 1)], BF16, name=f"vg{t}", tag=f"vg{t}")
                  for t in range(16)]
            ot = [pp.tile([128, S], BF16, name=f"ot{p}", tag=f"ot{p}")
                  for p in range(NP)]
            bq_sb = pp.tile([128, NP], F32, name="bq_sb", tag="bq_sb")
            bk_sb = pp.tile([128, NP], F32, name="bk_sb", tag="bk_sb")
            bvb_sb = pp.tile([128, HD2], F32, name="bvb_sb", tag="bvb_sb")
            bob_sb = pp.tile([128, D], F32, name="bob_sb", tag="bob_sb")

            # DMA issue order = priority order. HWDGE setup is ~0.6us per
            # dma_start on the issuing sequencer, so balance the 56 input
            # DMAs across BOTH capable engines (sync + scalar) or the issue
            # serialization itself (~24us) becomes the prologue bottleneck.
            # pairwise wq[d]+xt[d] issue: matmul d of the first projection
            # group can start as soon as ITS two tiles land, so the PE
            # warms up during the load phase instead of after it
            for d in range(8):
                nc.sync.dma_start(wq_sb[d][:], wqh[128 * d:128 * (d + 1), :])
                nc.sync.dma_start(xt_sb[d][:, 0:512], xT[128 * d:128 * (d + 1), 0:512])
                nc.scalar.dma_start(wk_sb[d][:], wkh[128 * d:128 * (d + 1), :])
                nc.scalar.dma_start(xt_sb[d][:, 1024:1536],
                                    xT[128 * d:128 * (d + 1), 1024:1536])
            for d in range(8):
                nc.sync.dma_start(xt_sb[d][:, 512:1024],
                                  xT[128 * d:128 * (d + 1), 512:1024])
                nc.scalar.dma_start(xt_sb[d][:, 1536:2048],
                                    xT[128 * d:128 * (d + 1), 1536:2048])
            for d in range(8):
                nc.scalar.dma_start(wv_sb[d][:], wvh[128 * d:128 * (d + 1), :])
            # tiny biases early on sync; the big late-needed tensors (bvb,
            # bob, wo) at the END of scalar's queue so they don't steal
            # early DMA bandwidth from the critical wq/wk/xt loads (gpsimd
            # issues start first and would front-run them).
            nc.sync.dma_start(bq_sb[:], bq2[:])
            nc.sync.dma_start(bk_sb[:], bk2[:])
            nc.scalar.dma_start(bvb_sb[:], bvb[:])
            nc.scalar.dma_start(bob_sb[:], bob[:])
            for d in range(4):
                nc.scalar.dma_start(wo_sb[d][:], woh[128 * d:128 * (d + 1), :])

            def proj_group(w_sb, p, qh, dst, b_sb):
                """One [128,1024] PSUM group: token cols 1024*qh..+1024 of the
                pair-p projection, written transposed into dst (qt/kt).
                Returns the written AP (used as a scheduling guard)."""
                ps = stp.tile([128, 1024], F32, name="psp", tag="st", bufs=2)
                for half in range(2):
                    qs = slice(1024 * qh + 512 * half, 1024 * qh + 512 * (half + 1))
                    for d in range(8):
                        nc.tensor.matmul(
                            ps[:, 512 * half:512 * (half + 1)],
                            w_sb[d][:, 128 * p:128 * (p + 1)],
                            xt_sb[d][:, qs],
                            start=(d == 0), stop=(d == 7))
                out_ap = dst[:, 1024 * qh:1024 * (qh + 1)]
                nc.vector.tensor_scalar_add(out_ap, ps[:], b_sb[:, p:p + 1])
                return out_ap

            def v_group(g):
                """V projection for token chunks ti=2g, 2g+1 in one PSUM tile."""
                ps = stp.tile([128, 1024], F32, name="psv", tag="st", bufs=2)
                for half in range(2):
                    ti = 2 * g + half
                    for d in range(8):
                        nc.tensor.matmul(
                            ps[:, 512 * half:512 * (half + 1)],
                            xt_sb[d][:, 128 * ti:128 * (ti + 1)],
                            wv_sb[d][:, :],
                            start=(d == 0), stop=(d == 7))
                for half in range(2):
                    ti = 2 * g + half
                    dst = vg[ti][:].rearrange("p (h w) -> p h w", w=65)[:, :, 0:64]
                    nc.vector.tensor_add(
                        dst,
                        ps[:, 512 * half:512 * (half + 1)].rearrange(
                            "p (h w) -> p h w", w=64),
                        bvb_sb[:].rearrange("p (h w) -> p h w", w=64))
                    nc.vector.memset(
                        vg[ti][:].rearrange("p (h w) -> p h w", w=65)[:, :, 64:65],
                        1.0)

            def o_group(t8, split=1):
                """Output projection for token rows 128*t8..+128 (partial over
                this core's 512 head dims), + bias, DMA to DRAM. The tail
                groups pass split=2 so the final transfers drain on two
                queues each."""
                ps = stp.tile([128, 1024], F32, name="pso", tag="st", bufs=2)
                for hf in range(2):
                    for pp_ in range(NP):
                        nc.tensor.matmul(
                            ps[:, 512 * hf:512 * (hf + 1)],
                            ot[pp_][:, 128 * t8:128 * (t8 + 1)],
                            wo_sb[pp_][:, 512 * hf:512 * (hf + 1)],
                            start=(pp_ == 0), stop=(pp_ == NP - 1))
                osb = op_.tile([128, D], BF16, name="osb", tag="osb", bufs=2)
                nc.vector.tensor_add(osb[:], ps[:], bob_sb[:])
                for c in range(split):
                    cs = slice(D // split * c, D // split * (c + 1))
                    nc.sync.dma_start(out[128 * t8:128 * (t8 + 1), cs],
                                      osb[:, cs])

            # Q/K projections for pairs 0 and 1 up front (first tensor work).
            qt = [None] * NP
            kt = [None] * NP

            def qk_proj_groups(p):
                """Return the 4 emission thunks of pair p's Q+K projections."""
                qt[p] = qkp.tile([128, S], BF16, name=f"qt_p{p}", tag="qt", bufs=2)
                kt[p] = qkp.tile([128, S], BF16, name=f"kt_p{p}", tag="kt", bufs=2)
                return [
                    lambda qh=qh: proj_group(wq_sb, p, qh, qt[p], bq_sb)
                    for qh in range(2)
                ] + [
                    lambda qh=qh: proj_group(wk_sb, p, qh, kt[p], bk_sb)
                    for qh in range(2)
                ]

            for g in qk_proj_groups(0):
                g()
            pending1 = qk_proj_groups(1)
            for g in pending1[:2]:
                g()

            def finish_norm(p, qb, po, rzb):
                """Broadcast 1/Z down 64 partitions (tiny bf16 ones
                outer-product matmuls into an st-pool PSUM tile), copy to
                SBUF, multiply. Emitted ~2 k-iterations into the NEXT
                attention block so the tensor queue never waits on the
                reciprocal."""
                qs = slice(512 * qb, 512 * (qb + 1))
                pb = stp.tile([128, 1024], F32, name="pb", tag="st", bufs=2)
                nc.tensor.matmul(pb[0:64, 0:512], ones_sb[0:1, :],
                                 rzb[0:1, 0:512], start=True, stop=True)
                nc.tensor.matmul(pb[0:64, 512:1024], ones_sb[32:33, :],
                                 rzb[32:33, 512:1024], start=True, stop=True)
                rb = rzp.tile([64, 1024], F32, name="rb", tag="rb", bufs=2)
                nc.vector.tensor_copy(rb[:], pb[0:64, :])
                nc.vector.tensor_mul(ot[p][0:64, qs], po[0:64, 0:512],
                                     rb[0:64, 0:512])
                nc.vector.tensor_mul(ot[p][64:128, qs], po[0:64, 512:1024],
                                     rb[0:64, 512:1024])

            def attention(p, qb, filler, norm_prev):
                """Attention for pair p, query block qb (512 queries).
                `filler`: list of (k_slot, thunk) emission thunks
                (projection / output groups) interleaved into the k-loop to
                keep the tensor queue fed.
                `norm_prev`: (recip_thunk, finish_thunk) of the previous
                block. The reciprocal goes at k==3 so it sits BEHIND this
                block's projection bias-add on the in-order DVE queue (a
                recip emitted earlier delays that add and stalls the tensor
                engine on the st-pool slot); the broadcast+multiply go at
                k==12, by which time the reciprocal is long done.
                Returns this block's (recip_thunk, finish_thunk)."""
                c0 = 130 * p
                c1 = 130 * p + 65
                qs = slice(512 * qb, 512 * (qb + 1))
                po = pop.tile([128, 1024], F32, name="po", tag="po", bufs=2)
                filler = sorted(filler, key=lambda x: x[0])
                fi = 0
                guard = None
                for k in range(16):
                    # finish_norm MUST precede same-slot fillers (the p==3
                    # output-projection groups read ot written by its muls)
                    if norm_prev is not None and k == 12:
                        norm_prev[1]()
                    while fi < len(filler) and filler[fi][0] == k:
                        r = filler[fi][1]()
                        if r is not None:
                            guard = r
                        fi += 1
                    # ... while the reciprocal guard needs the k<=3 filler
                    # (projection group) emitted first
                    if norm_prev is not None and k == 3:
                        norm_prev[0](guard)
                    ks = slice(128 * k, 128 * (k + 1))
                    st = stp.tile([128, 1024], F32, name="st", tag="st", bufs=2)
                    nc.tensor.matmul(st[:, 0:512], kt[p][0:64, ks], qt[p][0:64, qs],
                                     start=True, stop=True)
                    nc.tensor.matmul(st[:, 512:1024], kt[p][64:128, ks],
                                     qt[p][64:128, qs], start=True, stop=True)
                    pt = ptp.tile([128, 1024], BF16, name="pt", tag="pt", bufs=4)
                    nc.scalar.activation(pt[:], st[:], EXP, scale=0.125)
                    first, last = (k == 0), (k == 15)
                    nc.tensor.matmul(po[0:65, 0:512], vg[k][:, c0:c0 + 65],
                                     pt[:, 0:512], start=first, stop=last)
                    nc.tensor.matmul(po[0:65, 512:1024], vg[k][:, c1:c1 + 65],
                                     pt[:, 512:1024], start=first, stop=last)
                while fi < len(filler):
                    filler[fi][1]()
                    fi += 1
                rz = rzp.tile([33, 1024], F32, name="rz", tag="rz", bufs=2)
                rzb = rzp.tile([33, 1024], BF16, name="rzb", tag="rzb", bufs=2)

                def recip_thunk(guard_ap=None):
                    # The per-engine schedule is STATIC, ordered by
                    # dependency readiness -- the ~7us reciprocal chain
                    # would get placed ahead of the co-scheduled projection
                    # group's PSUM drain on DVE, and the held st-pool slot
                    # then stalls the tensor engine ~6us. Guard: copy the
                    # drained projection row into po's unused PSUM row 96
                    # (partition bases must be 32-aligned); each recip chunk
                    # reads rows 64:97, giving every chunk a clean RAW
                    # dependency on the drain. (rz rows 1..32 are
                    # recip-of-garbage and never read.)
                    with nc.allow_low_precision(reason="softmax 1/Z"):
                        if guard_ap is not None:
                            nc.vector.tensor_copy(po[96:97, :],
                                                  guard_ap[0:1, :])
                        for i in range(8):
                            cs = slice(128 * i, 128 * (i + 1))
                            if guard_ap is not None:
                                nc.vector.reciprocal(rz[0:33, cs],
                                                     po[64:97, cs])
                            else:
                                nc.vector.reciprocal(rz[0:1, cs],
                                                     po[64:65, cs])
                    nc.vector.tensor_copy(rzb[0:1, 0:512], rz[0:1, 0:512])
                    nc.vector.tensor_copy(rzb[32:33, 512:1024],
                                          rz[0:1, 512:1024])

                return (recip_thunk, lambda: finish_norm(p, qb, po, rzb))

            # pair 0, query block 0: interleave the 8 V-projection groups
            # (V group g covers key chunks 2g,2g+1, always ahead of PV k=g).
            norm = attention(0, 0,
                             [(2 * g, lambda g=g: v_group(g)) for g in range(8)],
                             None)
            # pair 0, blocks 1-3: finish pair 1's projections early
            norm = attention(0, 1, [(0, pending1[2]), (8, pending1[3])], norm)
            norm = attention(0, 2, [], norm)
            norm = attention(0, 3, [], norm)
            # pairs 1..3: interleave next pair's projections per query block
            for p in range(1, NP):
                nxt = qk_proj_groups(p + 1) if p + 1 < NP else []
                for qb in range(4):
                    filler = [(0, nxt[qb])] if qb < len(nxt) else []
                    # last pair: interleave output projection for finished
                    # query blocks (needs all pairs => only valid on p==3;
                    # slots 12+ put them after the deferred normalize)
                    if p == NP - 1 and qb > 0:
                        filler = [(12 + i, lambda t8=t8: o_group(t8))
                                  for i, t8 in enumerate(range(4 * (qb - 1), 4 * qb))]
                    norm = attention(p, qb, filler, norm)
            # tail: final normalize + output projection for the last block
            norm[0]()
            norm[1]()
            for t8 in range(12, 16):
                o_group(t8, split=2)

    split_multi_waits(nc)
    return nc


_CACHED_NC = None


def get_program():
    global _CACHED_NC
    if _CACHED_NC is None:
        _CACHED_NC = build_program()
    return _CACHED_NC


def make_in_maps(x, Wq, bq, Wk, bk, Wv, bv, Wo, bo):
    x = np.asarray(x, np.float32)
    bf = ml_dtypes.bfloat16
    wqT = np.ascontiguousarray(np.asarray(Wq, np.float32).T)
    wkT = np.ascontiguousarray(np.asarray(Wk, np.float32).T)
    wvT = np.ascontiguousarray(np.asarray(Wv, np.float32).T)
    woT = np.ascontiguousarray(np.asarray(Wo, np.float32).T)
    bq = np.asarray(bq, np.float32)
    bk = np.asarray(bk, np.float32)
    bv = np.asarray(bv, np.float32)
    bo = np.asarray(bo, np.float32)
    in_maps = []
    for c in range(NCORES):
        b, hh = c // 2, c % 2
        hs = slice(HD2 * hh, HD2 * (hh + 1))
        m = {
            "xT": np.ascontiguousarray(x[b].T).astype(bf),
            "wqh": np.ascontiguousarray(wqT[:, hs]).astype(bf),
            "wkh": np.ascontiguousarray(wkT[:, hs]).astype(bf),
            "wvh": np.ascontiguousarray(wvT[:, hs]).astype(bf),
            "woh": np.ascontiguousarray(woT[hs, :]).astype(bf),
            "bq2": np.ascontiguousarray(bq[hs].reshape(NP, 128).T),
            "bk2": np.ascontiguousarray(bk[hs].reshape(NP, 128).T),
            "bvb": np.ascontiguousarray(np.tile(bv[hs], (128, 1))),
            "bob": np.ascontiguousarray(
                np.tile(bo if hh == 0 else np.zeros_like(bo), (128, 1))),
            "ones2": np.ones((2, 64), ml_dtypes.bfloat16),
            "tag": np.zeros((1, VERSION), np.float32),
        }
        in_maps.append(m)
    return in_maps


def kernel(x, Wq, bq, Wk, bk, Wv, bv, Wo, bo):
    nc = get_program()
    in_maps = make_in_maps(x, Wq, bq, Wk, bk, Wv, bv, Wo, bo)
    res = run_bass_kernel_spmd(nc, in_maps, list(range(NCORES)))
    out = np.empty((4, S, D), np.float32)
    for b in range(4):
        out[b] = np.asarray(res.results[2 * b]["out"], np.float32)
        out[b] += np.asarray(res.results[2 * b + 1]["out"], np.float32)
    return out


# revision 80
# speedup vs baseline: 1.0107x; 1.0040x over previous
"""Multi-head attention (dense transformer block) on 8 Trainium2 NeuronCores.

Sharding: (batch=4) x (head-half=2) -> 8 shards. Each core computes, for its
batch element b and its 8 heads (heads 8*hh .. 8*hh+7), the FULL 2048-query
attention plus a PARTIAL output projection (contraction over its 512 head
dims); the host-side gather sums the two partials per batch element (the
output projection is linear; the bias is folded into the hh=0 core only).
This removes the redundant K/V projection compute the pure data-parallel
split required, with zero cross-core communication.

Per core:
  Qt_p/Kt_p = (W @ x.T)[pair rows]  (transposed, 128 x 2048) for 4 head-pairs
  V         = x @ Wv.T (natural layout, per-head 65-col interleave with a
              trailing ones column for the softmax denominator)
  attention per pair p, query-block qb (512 q), key-chunk k (128 keys):
    St for both heads in one 2-bank PSUM tile via a row-paired matmul pair,
    one wide exp(St/8) on ACT, and one M=65 matmul per head accumulates
    [V.T @ Pt ; ones.T @ Pt] so numerator and denominator come from the
    same instruction.
    normalize: 1/Z via a fast approximate DVE reciprocal; the 64-partition
    broadcast (tiny ones outer-product matmuls) for block qb is DEFERRED
    into the next block's k-loop, so the tensor queue never waits on the
    reciprocal (the baseline lost ~70us to that head-of-line stall).
  out_partial = Ot.T @ Wo[pair rows] + (bo if hh==0)   [2048 x 1024 f32]

Emission order pipelines the engines: V-projection PSUM groups interleave
into the first attention query block, Q/K projections for pair p+1
interleave per query block of pair p, and output-projection chunks
interleave into the last pair's attention, so the tensor engine never
idles waiting on ACT and vice versa.

Matmul operands are bf16 (fp32 PSUM accumulation).
"""

import numpy as np
import ml_dtypes

import concourse.bass as bass
import concourse.tile as tile
import concourse.mybir as mybir
from concourse.bass_utils import run_bass_kernel_spmd

F32 = mybir.dt.float32
F32R = mybir.dt.float32r
BF16 = mybir.dt.bfloat16
EXP = mybir.ActivationFunctionType.Exp
IDENT = mybir.ActivationFunctionType.Identity

D = 1024          # d_model
S = 2048          # sequence length
NH = 16           # heads total
DH = 64           # head dim
HD2 = 512         # head dims per core (8 heads)
NP = 4            # head-pairs per core
NCORES = 8
VERSION = 20       # bump on every program change: busts stale NEFF caches


def split_multi_waits(nc):
    """The walrus build in this container accepts at most one sync-wait per
    instruction; move extra waits onto same-engine nops inserted before the
    offending instruction."""
    k = 0
    for f in nc.m.functions:
        for bb in f.blocks:
            out, changed = [], False
            for inst in bb.instructions:
                si = inst.sync_info
                waits = list(si.on_wait) if si and si.on_wait else []
                if len(waits) > 1:
                    changed = True
                    for w in waits[:-1]:
                        nop = mybir.InstNoOp(name=f"wsplit-{k}", ins=[], outs=[])
                        k += 1
                        nop.engine = inst.engine
                        nop.sync_info = mybir.SyncInfo(on_wait=[w], on_update=[])
                        nc.register_instruction(nop, overwrite=True)
                        out.append(nop)
                    si.on_wait = waits[-1:]
                out.append(inst)
            if changed:
                bb.instructions = out


def build_program():
    nc = bass.Bass()
    xT = nc.declare_dram_parameter("xT", [D, S], BF16, isOutput=False)
    wqh = nc.declare_dram_parameter("wqh", [D, HD2], BF16, isOutput=False)
    wkh = nc.declare_dram_parameter("wkh", [D, HD2], BF16, isOutput=False)
    wvh = nc.declare_dram_parameter("wvh", [D, HD2], BF16, isOutput=False)
    woh = nc.declare_dram_parameter("woh", [HD2, D], BF16, isOutput=False)
    bq2 = nc.declare_dram_parameter("bq2", [128, NP], F32, isOutput=False)
    bk2 = nc.declare_dram_parameter("bk2", [128, NP], F32, isOutput=False)
    bvb = nc.declare_dram_parameter("bvb", [128, HD2], F32, isOutput=False)
    bob = nc.declare_dram_parameter("bob", [128, D], F32, isOutput=False)
    ones2 = nc.declare_dram_parameter("ones2", [2, 64], BF16, isOutput=False)
    tag = nc.declare_dram_parameter("tag", [1, VERSION], F32, isOutput=False)
    out = nc.declare_dram_parameter("out", [S, D], BF16, isOutput=True)

    with tile.TileContext(nc) as tc:
        with tc.tile_pool(name="persist", bufs=1) as pp, \
             tc.tile_pool(name="qk", bufs=2) as qkp, \
             tc.tile_pool(name="pt", bufs=4) as ptp, \
             tc.tile_pool(name="rz", bufs=2) as rzp, \
             tc.tile_pool(name="osb", bufs=2) as op_, \
             tc.tile_pool(name="psS", bufs=2, space="PSUM") as stp, \
             tc.tile_pool(name="psO", bufs=2, space="PSUM") as pop:
            tag_sb = pp.tile([1, VERSION], F32, name="tag_sb", tag="tag_sb")
            nc.sync.dma_start(tag_sb[:], tag[:])
            # ones rows at partitions 0 and 32 (matmul lhsT base must match
            # its rhs base; the 1/Z rows live at partitions 0 and 32)
            ones_sb = pp.tile([33, 64], BF16, name="ones_sb", tag="ones_sb")
            nc.sync.dma_start(ones_sb[0:1, :], ones2[0:1, :])
            nc.sync.dma_start(ones_sb[32:33, :], ones2[1:2, :])

            # resident tiles
            xt_sb = [pp.tile([128, S], BF16, name=f"xt{d}", tag=f"xt{d}")
                     for d in range(8)]
            wq_sb = [pp.tile([128, HD2], BF16, name=f"wq{d}", tag=f"wq{d}")
                     for d in range(8)]
            wk_sb = [pp.tile([128, HD2], BF16, name=f"wk{d}", tag=f"wk{d}")
                     for d in range(8)]
            wv_sb = [pp.tile([128, HD2], BF16, name=f"wv{d}", tag=f"wv{d}")
                     for d in range(8)]
            wo_sb = [pp.tile([128, D], BF16, name=f"wo{d}", tag=f"wo{d}")
                     for d in range(4)]
            vg = [pp.tile([128, 8 * (DH + 1)], BF16, name=f"vg{t}", tag=f"vg{t}")
                  for t in range(16)]
            ot = [pp.tile([128, S], BF16, name=f"ot{p}", tag=f"ot{p}")
                  for p in range(NP)]
            bq_sb = pp.tile([128, NP], F32, name="bq_sb", tag="bq_sb")
            bk_sb = pp.tile([128, NP], F32, name="bk_sb", tag="bk_sb")
            bvb_sb = pp.tile([128, HD2], F32, name="bvb_sb", tag="bvb_sb")
            bob_sb = pp.tile([128, D], F32, name="bob_sb", tag="bob_sb")

            # DMA issue order = priority order. HWDGE setup is ~0.6us per
            # dma_start on the issuing sequencer, so balance the 56 input
            # DMAs across BOTH capable engines (sync + scalar) or the issue
            # serialization itself (~24us) becomes the prologue bottleneck.
            for d in range(8):
                nc.sync.dma_start(wq_sb[d][:], wqh[128 * d:128 * (d + 1), :])
                nc.scalar.dma_start(wk_sb[d][:], wkh[128 * d:128 * (d + 1), :])
            for d in range(8):
                nc.sync.dma_start(xt_sb[d][:, 0:512], xT[128 * d:128 * (d + 1), 0:512])
                nc.scalar.dma_start(xt_sb[d][:, 1024:1536],
                                    xT[128 * d:128 * (d + 1), 1024:1536])
            for d in range(8):
                nc.sync.dma_start(xt_sb[d][:, 512:1024],
                                  xT[128 * d:128 * (d + 1), 512:1024])
                nc.scalar.dma_start(xt_sb[d][:, 1536:2048],
                                    xT[128 * d:128 * (d + 1), 1536:2048])
            for d in range(8):
                nc.scalar.dma_start(wv_sb[d][:], wvh[128 * d:128 * (d + 1), :])
            # tiny biases early on sync; the big late-needed tensors (bvb,
            # bob, wo) at the END of scalar's queue so they don't steal
            # early DMA bandwidth from the critical wq/wk/xt loads (gpsimd
            # issues start first and would front-run them).
            nc.sync.dma_start(bq_sb[:], bq2[:])
            nc.sync.dma_start(bk_sb[:], bk2[:])
            nc.scalar.dma_start(bvb_sb[:], bvb[:])
            nc.scalar.dma_start(bob_sb[:], bob[:])
            for d in range(4):
                nc.scalar.dma_start(wo_sb[d][:], woh[128 * d:128 * (d + 1), :])

            def proj_group(w_sb, p, qh, dst, b_sb):
                """One [128,1024] PSUM group: token cols 1024*qh..+1024 of the
                pair-p projection, written transposed into dst (qt/kt).
                Returns the written AP (used as a scheduling guard)."""
                ps = stp.tile([128, 1024], F32, name="psp", tag="st", bufs=2)
                for half in range(2):
                    qs = slice(1024 * qh + 512 * half, 1024 * qh + 512 * (half + 1))
                    for d in range(8):
                        nc.tensor.matmul(
                            ps[:, 512 * half:512 * (half + 1)],
                            w_sb[d][:, 128 * p:128 * (p + 1)],
                            xt_sb[d][:, qs],
                            start=(d == 0), stop=(d == 7))
                out_ap = dst[:, 1024 * qh:1024 * (qh + 1)]
                nc.vector.tensor_scalar_add(out_ap, ps[:], b_sb[:, p:p + 1])
                return out_ap

            def v_group(g):
                """V projection for token chunks ti=2g, 2g+1 in one PSUM tile."""
                ps = stp.tile([128, 1024], F32, name="psv", tag="st", bufs=2)
                for half in range(2):
                    ti = 2 * g + half
                    for d in range(8):
                        nc.tensor.matmul(
                            ps[:, 512 * half:512 * (half + 1)],
                            xt_sb[d][:, 128 * ti:128 * (ti + 1)],
                            wv_sb[d][:, :],
                            start=(d == 0), stop=(d == 7))
                for half in range(2):
                    ti = 2 * g + half
                    dst = vg[ti][:].rearrange("p (h w) -> p h w", w=65)[:, :, 0:64]
                    nc.vector.tensor_add(
                        dst,
                        ps[:, 512 * half:512 * (half + 1)].rearrange(
                            "p (h w) -> p h w", w=64),
                        bvb_sb[:].rearrange("p (h w) -> p h w", w=64))
                    nc.vector.memset(
                        vg[ti][:].rearrange("p (h w) -> p h w", w=65)[:, :, 64:65],
                        1.0)

            def o_group(t8, split=1):
                """Output projection for token rows 128*t8..+128 (partial over
                this core's 512 head dims), + bias, DMA to DRAM. The tail
                groups pass split=2 so the final transfers drain on two
                queues each."""
                ps = stp.tile([128, 1024], F32, name="pso", tag="st", bufs=2)
                for hf in range(2):
                    for pp_ in range(NP):
                        nc.tensor.matmul(
                            ps[:, 512 * hf:512 * (hf + 1)],
                            ot[pp_][:, 128 * t8:128 * (t8 + 1)],
                            wo_sb[pp_][:, 512 * hf:512 * (hf + 1)],
                            start=(pp_ == 0), stop=(pp_ == NP - 1))
                osb = op_.tile([128, D], BF16, name="osb", tag="osb", bufs=2)
                nc.vector.tensor_add(osb[:], ps[:], bob_sb[:])
                for c in range(split):
                    cs = slice(D // split * c, D // split * (c + 1))
                    nc.sync.dma_start(out[128 * t8:128 * (t8 + 1), cs],
                                      osb[:, cs])

            # Q/K projections for pairs 0 and 1 up front (first tensor work).
            qt = [None] * NP
            kt = [None] * NP

            def qk_proj_groups(p):
                """Return the 4 emission thunks of pair p's Q+K projections."""
                qt[p] = qkp.tile([128, S], BF16, name=f"qt_p{p}", tag="qt", bufs=2)
                kt[p] = qkp.tile([128, S], BF16, name=f"kt_p{p}", tag="kt", bufs=2)
                return [
                    lambda qh=qh: proj_group(wq_sb, p, qh, qt[p], bq_sb)
                    for qh in range(2)
                ] + [
                    lambda qh=qh: proj_group(wk_sb, p, qh, kt[p], bk_sb)
                    for qh in range(2)
                ]

            for g in qk_proj_groups(0):
                g()
            pending1 = qk_proj_groups(1)
            for g in pending1[:2]:
                g()

            def finish_norm(p, qb, po, rzb):
                """Broadcast 1/Z down 64 partitions (tiny bf16 ones
                outer-product matmuls into an st-pool PSUM tile), copy to
                SBUF, multiply. Emitted ~2 k-iterations into the NEXT
                attention block so the tensor queue never waits on the
                reciprocal."""
                qs = slice(512 * qb, 512 * (qb + 1))
                pb = stp.tile([128, 1024], F32, name="pb", tag="st", bufs=2)
                nc.tensor.matmul(pb[0:64, 0:512], ones_sb[0:1, :],
                                 rzb[0:1, 0:512], start=True, stop=True)
                nc.tensor.matmul(pb[0:64, 512:1024], ones_sb[32:33, :],
                                 rzb[32:33, 512:1024], start=True, stop=True)
                rb = rzp.tile([64, 1024], F32, name="rb", tag="rb", bufs=2)
                nc.vector.tensor_copy(rb[:], pb[0:64, :])
                nc.vector.tensor_mul(ot[p][0:64, qs], po[0:64, 0:512],
                                     rb[0:64, 0:512])
                nc.vector.tensor_mul(ot[p][64:128, qs], po[0:64, 512:1024],
                                     rb[0:64, 512:1024])

            def attention(p, qb, filler, norm_prev):
                """Attention for pair p, query block qb (512 queries).
                `filler`: list of (k_slot, thunk) emission thunks
                (projection / output groups) interleaved into the k-loop to
                keep the tensor queue fed.
                `norm_prev`: (recip_thunk, finish_thunk) of the previous
                block. The reciprocal goes at k==3 so it sits BEHIND this
                block's projection bias-add on the in-order DVE queue (a
                recip emitted earlier delays that add and stalls the tensor
                engine on the st-pool slot); the broadcast+multiply go at
                k==12, by which time the reciprocal is long done.
                Returns this block's (recip_thunk, finish_thunk)."""
                c0 = 130 * p
                c1 = 130 * p + 65
                qs = slice(512 * qb, 512 * (qb + 1))
                po = pop.tile([128, 1024], F32, name="po", tag="po", bufs=2)
                filler = sorted(filler, key=lambda x: x[0])
                fi = 0
                guard = None
                for k in range(16):
                    # finish_norm MUST precede same-slot fillers (the p==3
                    # output-projection groups read ot written by its muls)
                    if norm_prev is not None and k == 12:
                        norm_prev[1]()
                    while fi < len(filler) and filler[fi][0] == k:
                        r = filler[fi][1]()
                        if r is not None:
                            guard = r
                        fi += 1
                    # ... while the reciprocal guard needs the k<=3 filler
                    # (projection group) emitted first
                    if norm_prev is not None and k == 3:
                        norm_prev[0](guard)
                    ks = slice(128 * k, 128 * (k + 1))
                    st = stp.tile([128, 1024], F32, name="st", tag="st", bufs=2)
                    nc.tensor.matmul(st[:, 0:512], kt[p][0:64, ks], qt[p][0:64, qs],
                                     start=True, stop=True)
                    nc.tensor.matmul(st[:, 512:1024], kt[p][64:128, ks],
                                     qt[p][64:128, qs], start=True, stop=True)
                    pt = ptp.tile([128, 1024], BF16, name="pt", tag="pt", bufs=4)
                    nc.scalar.activation(pt[:], st[:], EXP, scale=0.125)
                    first, last = (k == 0), (k == 15)
                    nc.tensor.matmul(po[0:65, 0:512], vg[k][:, c0:c0 + 65],
                                     pt[:, 0:512], start=first, stop=last)
                    nc.tensor.matmul(po[0:65, 512:1024], vg[k][:, c1:c1 + 65],
                                     pt[:, 512:1024], start=first, stop=last)
                while fi < len(filler):
                    filler[fi][1]()
                    fi += 1
                rz = rzp.tile([33, 1024], F32, name="rz", tag="rz", bufs=2)
                rzb = rzp.tile([33, 1024], BF16, name="rzb", tag="rzb", bufs=2)

                def recip_thunk(guard_ap=None):
                    # The per-engine schedule is STATIC, ordered by
                    # dependency readiness -- the ~7us reciprocal chain
                    # would get placed ahead of the co-scheduled projection
                    # group's PSUM drain on DVE, and the held st-pool slot
                    # then stalls the tensor engine ~6us. Guard: copy the
                    # drained projection row into po's unused PSUM row 96
                    # (partition bases must be 32-aligned); each recip chunk
                    # reads rows 64:97, giving every chunk a clean RAW
                    # dependency on the drain. (rz rows 1..32 are
                    # recip-of-garbage and never read.)
                    with nc.allow_low_precision(reason="softmax 1/Z"):
                        if guard_ap is not None:
                            nc.vector.tensor_copy(po[96:97, :],
                                                  guard_ap[0:1, :])
                        for i in range(8):
                            cs = slice(128 * i, 128 * (i + 1))
                            if guard_ap is not None:
                                nc.vector.reciprocal(rz[0:33, cs],
                                                     po[64:97, cs])
                            else:
                                nc.vector.reciprocal(rz[0:1, cs],
                                                     po[64:65, cs])
                    nc.vector.tensor_copy(rzb[0:1, 0:512], rz[0:1, 0:512])
                    nc.vector.tensor_copy(rzb[32:33, 512:1024],
                                          rz[0:1, 512:1024])

                return (recip_thunk, lambda: finish_norm(p, qb, po, rzb))

            # pair 0, query block 0: interleave the 8 V-projection groups
            # (V group g covers key chunks 2g,2g+1, always ahead of PV k=g).
            norm = attention(0, 0,
                             [(2 * g, lambda g=g: v_group(g)) for g in range(8)],
                             None)
            # pair 0, blocks 1-3: finish pair 1's projections early
            norm = attention(0, 1, [(0, pending1[2]), (8, pending1[3])], norm)
            norm = attention(0, 2, [], norm)
            norm = attention(0, 3, [], norm)
            # pairs 1..3: interleave next pair's projections per query block
            for p in range(1, NP):
                nxt = qk_proj_groups(p + 1) if p + 1 < NP else []
                for qb in range(4):
                    filler = [(0, nxt[qb])] if qb < len(nxt) else []
                    # last pair: interleave output projection for finished
                    # query blocks (needs all pairs => only valid on p==3;
                    # slots 12+ put them after the deferred normalize)
                    if p == NP - 1 and qb > 0:
                        filler = [(12 + i, lambda t8=t8: o_group(t8))
                                  for i, t8 in enumerate(range(4 * (qb - 1), 4 * qb))]
                    norm = attention(p, qb, filler, norm)
            # tail: final normalize + output projection for the last block
            norm[0]()
            norm[1]()
            for t8 in range(12, 16):
                o_group(t8, split=2)

    split_multi_waits(nc)
    return nc


_CACHED_NC = None


def get_program():
    global _CACHED_NC
    if _CACHED_NC is None:
        _CACHED_NC = build_program()
    return _CACHED_NC


def make_in_maps(x, Wq, bq, Wk, bk, Wv, bv, Wo, bo):
    x = np.asarray(x, np.float32)
    bf = ml_dtypes.bfloat16
    wqT = np.ascontiguousarray(np.asarray(Wq, np.float32).T)
    wkT = np.ascontiguousarray(np.asarray(Wk, np.float32).T)
    wvT = np.ascontiguousarray(np.asarray(Wv, np.float32).T)
    woT = np.ascontiguousarray(np.asarray(Wo, np.float32).T)
    bq = np.asarray(bq, np.float32)
    bk = np.asarray(bk, np.float32)
    bv = np.asarray(bv, np.float32)
    bo = np.asarray(bo, np.float32)
    in_maps = []
    for c in range(NCORES):
        b, hh = c // 2, c % 2
        hs = slice(HD2 * hh, HD2 * (hh + 1))
        m = {
            "xT": np.ascontiguousarray(x[b].T).astype(bf),
            "wqh": np.ascontiguousarray(wqT[:, hs]).astype(bf),
            "wkh": np.ascontiguousarray(wkT[:, hs]).astype(bf),
            "wvh": np.ascontiguousarray(wvT[:, hs]).astype(bf),
            "woh": np.ascontiguousarray(woT[hs, :]).astype(bf),
            "bq2": np.ascontiguousarray(bq[hs].reshape(NP, 128).T),
            "bk2": np.ascontiguousarray(bk[hs].reshape(NP, 128).T),
            "bvb": np.ascontiguousarray(np.tile(bv[hs], (128, 1))),
            "bob": np.ascontiguousarray(
                np.tile(bo if hh == 0 else np.zeros_like(bo), (128, 1))),
            "ones2": np.ones((2, 64), ml_dtypes.bfloat16),
            "tag": np.zeros((1, VERSION), np.float32),
        }
        in_maps.append(m)
    return in_maps


def kernel(x, Wq, bq, Wk, bk, Wv, bv, Wo, bo):
    nc = get_program()
    in_maps = make_in_maps(x, Wq, bq, Wk, bk, Wv, bv, Wo, bo)
    res = run_bass_kernel_spmd(nc, in_maps, list(range(NCORES)))
    out = np.empty((4, S, D), np.float32)
    for b in range(4):
        out[b] = np.asarray(res.results[2 * b]["out"], np.float32)
        out[b] += np.asarray(res.results[2 * b + 1]["out"], np.float32)
    return out
